# revision 5
# baseline (speedup 1.0000x reference)
"""Multi-head transposed (channel) attention kernel for Trainium2.

Reference computation (per batch b, head h, c=32 channels, n=65536 spatial):
    q,k,v = split(qkv)                       # each [32, n] per (b,h)
    qh = q / max(||q||_row, 1e-12)           # L2 normalize over n
    kh = k / max(||k||_row, 1e-12)
    S = (qh @ kh.T) * temperature[h]         # [32, 32]
    A = softmax(S, axis=-1)
    out = A @ v                              # [32, n]

Sharding: 24 (b,h) pairs over 8 cores = 3 pairs/core, stacked on 96
partitions.

Host-side: q,k are L2-normalized on the host (temperature folded into q),
scaled by 256 and cast to fp8 e4m3, then pre-transposed into the SBUF tile
layout [chunk, 128 (spatial), sub, 192 (q|k channels)] so pass-1 loads are
fully contiguous plain DMAs.  v is cast to fp16.  The output is produced in
fp16 and upcast on the host.

Device (single continuous pipeline, DMA never idles):
  pass 1: stream qk tiles; per pair of 128-spatial subs, ONE fp8 DoubleRow
          matmul accumulates S^T = k^T-stack . q-stack into one PSUM bank
          (96x96, computed transposed so the chain needs no PE transpose).
          PE work here is ~15-30us inside a ~35us DMA window.
  chain:  exp((S^T)/65536) on the 3 diagonal 32x32 blocks -> block-diagonal
          fp16 attn^T (E); softmax denominators via E^T.T @ ones matmul
          (directly onto c partitions); reciprocal.  ~2us, overlapped with
          the v-load DMA stream that follows qk on the sync queue.
  pass 2: out = E.T @ v in fp16 N=512 matmuls (lhsT = E block-diagonal);
          PSUM->SBUF copies fold the 1/rowsum scale, alternating DVE/ACT;
          out stores issued from gpsimd to keep sync/scalar queues free.
"""

import ml_dtypes
import numpy as np

import concourse.bass as bass
import concourse.tile as tile
from concourse import bacc, mybir
from concourse.bass_utils import run_bass_kernel_spmd

F32 = mybir.dt.float32
F16 = mybir.dt.float16
F8 = mybir.dt.float8e4

B = 4
HD = 6
CH = 32          # channels per head
HW = 65536       # spatial size (256*256)
P = 96           # partition stack: 3 pairs * 32 channels
P2 = 192         # q-stack + k-stack channels
N_CORES = 8
PAIRS_PER_CORE = 3
QSCALE = 256.0   # host scale on normalized q,k before fp8 cast

FT = 4096        # pass-1 qk chunk (spatial)
NCH1 = HW // FT  # 16
SUB = 128
NSUB = FT // SUB  # 32
F2 = 4096        # pass-2 v-load / out-store chunk
NF = 512         # matmul free size (one PSUM bank)
NMM2 = F2 // NF  # 8
NCH2 = HW // F2  # 16

USE_DOUBLE_ROW = True
PASS2_FILLERS = 2


def build_nc():
    nc = bacc.Bacc("TRN2", target_bir_lowering=False, debug=False,
                   num_devices=N_CORES)
    qk_d = nc.dram_tensor("qk", [NCH1, SUB, NSUB, P2], F8,
                          kind="ExternalInput").ap()
    v_d = nc.dram_tensor("v", [P, HW], F16, kind="ExternalInput").ap()
    o_d = nc.dram_tensor("out", [P, HW], F16, kind="ExternalOutput").ap()

    with tile.TileContext(nc) as tc:
        _body(nc, tc, qk_d, v_d, o_d)
    nc.compile()
    return nc


def _body(nc, tc, qk_d, v_d, o_d):
    Exp = mybir.ActivationFunctionType.Exp
    Copy = mybir.ActivationFunctionType.Copy
    DR = mybir.MatmulPerfMode.DoubleRow

    with (
        tc.tile_pool(name="const", bufs=1) as constp,
        tc.tile_pool(name="persist", bufs=1) as pp,
    ):
        ones96 = constp.tile([P, 1], F16)
        nc.gpsimd.memset(ones96[:, :], 1.0)
        E_sb = pp.tile([P, P], F16)
        nc.gpsimd.memset(E_sb[:, :], 0.0)
        rinv = pp.tile([P, 1], F32)
        # warm the ACT Exp table so the chain doesn't pay the table load
        warm = pp.tile([1, 1], F32)
        nc.vector.memset(warm[:, :], 0.0)
        nc.scalar.activation(out=warm[:, :], in_=warm[:, :], func=Exp)

        # one PSUM bank accumulates S^T [96, 96]
        psS_cm = tc.tile_pool(name="psS", bufs=1, space="PSUM")
        psS_p = psS_cm.__enter__()
        acc = psS_p.tile([P, P], F32)

        # ---------------- pass 1: S^T = sum_s kT_s^T @ qT_s ----------------
        def mm_steps(qkT, s0, s1, t):
            for sp in range(s0, s1, 2):
                first = (t == 0 and sp == 0)
                last = (t == NCH1 - 1 and sp == NSUB - 2)
                if USE_DOUBLE_ROW:
                    nc.tensor.matmul(
                        acc[:, :],
                        lhsT=qkT[:, sp:sp + 2, P:P2],
                        rhs=qkT[:, sp:sp + 2, 0:P],
                        start=first, stop=last, perf_mode=DR,
                        skip_group_check=True)
                else:
                    for s in (sp, sp + 1):
                        nc.tensor.matmul(
                            acc[:, :],
                            lhsT=qkT[:, s, P:P2],
                            rhs=qkT[:, s, 0:P],
                            start=(t == 0 and s == 0),
                            stop=(t == NCH1 - 1 and s == NSUB - 1),
                            skip_group_check=True)

        with tc.tile_pool(name="io1", bufs=6) as io1:
            # first tile split in 4 pieces so the PE starts sooner
            qkT0 = io1.tile([SUB, NSUB, P2], F8, tag="qkT")
            PIECE = NSUB // 4
            for pc in range(4):
                sl = slice(pc * PIECE, (pc + 1) * PIECE)
                nc.sync.dma_start(out=qkT0[:, sl, :], in_=qk_d[0, :, sl, :])
                mm_steps(qkT0, pc * PIECE, (pc + 1) * PIECE, 0)
            for t in range(1, NCH1):
                qkT = io1.tile([SUB, NSUB, P2], F8, tag="qkT")
                nc.sync.dma_start(out=qkT[:, :, :], in_=qk_d[t])
                mm_steps(qkT, 0, NSUB, t)

        # ---------------- v loads (queue behind qk on the sync ring) -------
        iov_cm = tc.tile_pool(name="iov", bufs=12)
        iov = iov_cm.__enter__()
        v_tiles = []
        for m in range(NCH2):
            vn = iov.tile([P, F2], F16, tag="vn")
            nc.sync.dma_start(out=vn[:, :], in_=v_d[:, m * F2:(m + 1) * F2])
            v_tiles.append(vn)

        # ---------------- softmax chain (no transposes needed) -------------
        with tc.tile_pool(name="psC", bufs=1, space="PSUM") as psC:
            for j in range(PAIRS_PER_CORE):
                blk = slice(CH * j, CH * (j + 1))
                nc.scalar.activation(out=E_sb[blk, blk], in_=acc[blk, blk],
                                     func=Exp, scale=1.0 / (QSCALE * QSCALE))
            # rowsum_c = sum_d E^T[d,c] via E_sb^T @ ones -> [96,1] on c
            rs_ps = psC.tile([P, 1], F32, tag="rs")
            nc.tensor.matmul(rs_ps[:, :], lhsT=E_sb[:, :], rhs=ones96[:, :],
                             start=True, stop=True)
            nc.vector.reciprocal(out=rinv[:, :], in_=rs_ps[:, :])

        # release the accumulator bank so pass 2 can use 8 PSUM banks
        psS_cm.__exit__(None, None, None)

        # ---------------- pass 2: out = attn @ v ----------------
        mult = mybir.AluOpType.mult
        with (
            tc.tile_pool(name="ioo", bufs=4) as ioo,
            tc.tile_pool(name="psO", bufs=7, space="PSUM") as psOp,
            tc.tile_pool(name="psF", bufs=1, space="PSUM") as psFp,
        ):
            fps = psFp.tile([P, NF], F32)
            for m in range(NCH2):
                vn = v_tiles[m]
                on = ioo.tile([P, F2], F16, tag="on")
                for i in range(NMM2):
                    msl = slice(i * NF, (i + 1) * NF)
                    o_ps = psOp.tile([P, NF], F32, tag="o")
                    nc.tensor.matmul(o_ps[:, :], lhsT=E_sb[:, :],
                                     rhs=vn[:, msl], start=True, stop=True)
                    if (NMM2 * m + i) % 2 == 0:
                        nc.vector.tensor_scalar(
                            out=on[:, msl], in0=o_ps[:, :],
                            scalar1=rinv[:, :], scalar2=None, op0=mult)
                    else:
                        nc.scalar.activation(out=on[:, msl], in_=o_ps[:, :],
                                             func=Copy, scale=rinv[:, :])
                nc.gpsimd.dma_start(out=o_d[:, m * F2:(m + 1) * F2],
                                    in_=on[:, :])
                # PE keep-warm between chunks: occupies otherwise-idle PE
                # slots so the HAM clock gate stays at 8/8 (results unused)
                for _ in range(PASS2_FILLERS):
                    nc.tensor.matmul(fps[:, :], lhsT=E_sb[:, :],
                                     rhs=vn[:, 0:NF], start=True, stop=True,
                                     skip_group_check=True)
        iov_cm.__exit__(None, None, None)


_NC_CACHE = {}


def _get_nc():
    if "nc" not in _NC_CACHE:
        _NC_CACHE["nc"] = build_nc()
    return _NC_CACHE["nc"]


def _shard_inputs(qkv, temperature):
    qkv = np.asarray(qkv)
    temp = np.asarray(temperature, dtype=np.float32).reshape(-1)
    C = HD * CH
    q = qkv[:, 0 * C:1 * C].reshape(B, HD, CH, HW)
    k = qkv[:, 1 * C:2 * C].reshape(B, HD, CH, HW)
    v = qkv[:, 2 * C:3 * C].reshape(B, HD, CH, HW)
    # L2-normalize on host; fold temperature into q; scale for fp8 range
    qs = q / np.maximum(np.linalg.norm(q, axis=-1, keepdims=True), 1e-12)
    ks = k / np.maximum(np.linalg.norm(k, axis=-1, keepdims=True), 1e-12)
    qs = qs * (QSCALE * temp[None, :, None, None])
    ks = ks * QSCALE
    in_maps = []
    for core in range(N_CORES):
        pairs = [divmod(p, HD) for p in
                 range(core * PAIRS_PER_CORE, (core + 1) * PAIRS_PER_CORE)]
        qq = np.concatenate([qs[b_, h_] for b_, h_ in pairs], axis=0)
        kk = np.concatenate([ks[b_, h_] for b_, h_ in pairs], axis=0)
        qks = np.concatenate([qq, kk], axis=0).astype(ml_dtypes.float8_e4m3)
        # pre-transpose to the SBUF tile layout [chunk, p, sub, ch]
        qks = np.ascontiguousarray(
            qks.reshape(P2, NCH1, NSUB, SUB).transpose(1, 3, 2, 0))
        vs = np.concatenate([v[b_, h_] for b_, h_ in pairs],
                            axis=0).astype(np.float16)
        in_maps.append({"qk": qks, "v": vs})
    return in_maps


def _gather_output(results):
    out = np.empty((B, HD, CH, HW), dtype=np.float32)
    for core in range(N_CORES):
        o = results[core]["out"]
        for j in range(PAIRS_PER_CORE):
            b_, h_ = divmod(core * PAIRS_PER_CORE + j, HD)
            out[b_, h_] = o[CH * j:CH * (j + 1)].astype(np.float32)
    return out.reshape(B, HD * CH, 256, 256)


def kernel(qkv, temperature):
    in_maps = _shard_inputs(qkv, temperature)
    nc = _get_nc()
    res = run_bass_kernel_spmd(nc, in_maps, list(range(N_CORES)))
    return _gather_output(res.results)


if __name__ == "__main__":
    rng = np.random.default_rng(0)
    qkv = rng.standard_normal((B, 576, 256, 256), dtype=np.float32)
    temp = np.ones((HD, 1, 1), dtype=np.float32)
    out = kernel(qkv=qkv, temperature=temp)
    print("out", out.shape, out.dtype, float(np.abs(out).max()))
